# revision 25
# baseline (speedup 1.0000x reference)
"""Trainium2 Bass kernel for nn_Attention_63513976373985.

Strategy: pure data-parallel over the batch dim B=64 across 8 NeuronCores
(8 batches per core, all params replicated, no collectives).

v3: software-pipelined schedule.
  - d2 transposed on the HOST, loaded with plain DMAs on 2 queues (no
    xbar DMA-transposes, PE starts ~2us in).
  - P-reduction (atts = P . us) is 4-way column-tiled: pat matmuls are
    emitted as 4-MM quads (heads 2k,2k+1 x ct) delayed two heads behind
    the zs/tanh producer so all four col-groups run concurrently on the
    PE with their inputs already in SBUF; a full-width [128,8] "comb"
    matmul folds the groups into atts [8,S]. Each group chain carries its
    own start/stop (the has_written clear of start=True covers only the
    addressed partition rows -- HW-verified).
  - batch b-1's softmax/vs tail is interleaved into batch b's head loop
    (exp under h=1, score-transposes under h=2, vs matmuls under h=3,
    V-slab transposes under h=4) so the PE never idles through the
    softmax latency chain and HAM stays at K=8/8.
  - d3 relu+bias on DVE (dual-op tensor_scalar); tv tanh one 1024-wide
    ACTIVATE; exp with accum_out; 1/Z folded into the vs eviction.

Everything fp16 (fp32 PSUM accumulate).
"""
import sys

if "/opt/trn_rl_repo" not in sys.path:
    sys.path.insert(0, "/opt/trn_rl_repo")

import numpy as np

H, F, C, S, B = 8, 512, 256, 512, 64
NCORES = 8
BLOC = B // NCORES  # 8
OUTF = 128

_CACHE = {}


def build_nc(debug=False, dbg_b=0):
    import concourse.bass as bass  # noqa: F401
    import concourse.mybir as mybir
    import concourse.tile as tile
    from concourse import bacc
    from contextlib import ExitStack

    f32 = mybir.dt.float32
    f16 = mybir.dt.float16
    AF = mybir.ActivationFunctionType
    ALU = mybir.AluOpType

    nc = bacc.Bacc("TRN2", target_bir_lowering=False, debug=False,
                   num_devices=NCORES)

    # ---- DRAM parameters (per-core shard shapes) ----
    xt_d = nc.dram_tensor("xt", [128, 4, BLOC, S], f16, kind="ExternalInput")
    w1_d = nc.dram_tensor("w1r", [128, 4, 2, 128], f16, kind="ExternalInput")
    wv_d = nc.dram_tensor("wvr", [128, 4, C], f16, kind="ExternalInput")
    wtop_d = nc.dram_tensor("wtopr", [128, H, 2, 2, 128], f16,
                            kind="ExternalInput")
    wbot_d = nc.dram_tensor("wbotr", [128, H, 2, 2, 128], f16,
                            kind="ExternalInput")
    wcc_d = nc.dram_tensor("wccr", [128, 2 * H, OUTF], f16,
                           kind="ExternalInput")
    # packed small consts: [0:8]=comb, [8:72]=pblk, [72:104]=d1t,
    # [104:112]=id8(rows 0:8), [112:120]=ones18(row 0), [120:248]=bcc(row 0)
    pk_d = nc.dram_tensor("packed", [128, 248], f16, kind="ExternalInput")
    b1c_d = nc.dram_tensor("b1c", [128, 2], f32, kind="ExternalInput")
    out_d = nc.dram_tensor("out", [BLOC, OUTF], f32, kind="ExternalOutput")
    if debug:
        dbg_d3t = nc.dram_tensor("dbg_d3t", [128, 2, S], f16,
                                 kind="ExternalOutput")
        dbg_tv = nc.dram_tensor("dbg_tv", [128, 4, C], f16,
                                kind="ExternalOutput")
        dbg_attsg = nc.dram_tensor("dbg_attsg", [128, S], f16,
                                   kind="ExternalOutput")
        dbg_esc = nc.dram_tensor("dbg_esc", [8, S], f16,
                                 kind="ExternalOutput")
        dbg_zc = nc.dram_tensor("dbg_zc", [128, 2, H, BLOC], f32,
                                kind="ExternalOutput")
        dbg_vs = nc.dram_tensor("dbg_vs", [BLOC, 8, C], f16,
                                kind="ExternalOutput")

    with tile.TileContext(nc) as tc, ExitStack() as stk:
        const = stk.enter_context(tc.tile_pool(name="const", bufs=1))
        xtp = stk.enter_context(tc.tile_pool(name="xtp", bufs=1))
        d3p = stk.enter_context(tc.tile_pool(name="d3p", bufs=2))
        tvpool = stk.enter_context(tc.tile_pool(name="tvpool", bufs=2))
        usp = stk.enter_context(tc.tile_pool(name="usp", bufs=4))
        smallsb = stk.enter_context(tc.tile_pool(name="smallsb", bufs=2))
        vpool = stk.enter_context(tc.tile_pool(name="vpool", bufs=1))
        pmm = stk.enter_context(tc.tile_pool(name="pmm", bufs=1, space="PSUM"))
        pzs = stk.enter_context(tc.tile_pool(name="pzs", bufs=2, space="PSUM"))
        patp = stk.enter_context(tc.tile_pool(name="patp", bufs=1,
                                              space="PSUM"))
        psmall = stk.enter_context(
            tc.tile_pool(name="psmall", bufs=1, space="PSUM"))

        # ---- DMA issue: 2 HWDGE queues (SP + Activation), few big DMAs.
        # (gpsimd.dma_start is SWDGE -- software descriptor generation with
        # ~8us startup and low throughput; avoid for everything.)
        xt_sb = xtp.tile([128, 4, BLOC, S], f16, tag="xt")
        w1_sb = const.tile([128, 4, 2, 128], f16, tag="w1")
        pk_sb = const.tile([128, 248], f16, tag="packed")
        b1c_sb = const.tile([128, 2], f32, tag="b1c")
        wv_sb = const.tile([128, 4, C], f16, tag="wv")
        wbot_sb = const.tile([128, H, 2, 2, 128], f16, tag="wbot")
        wtop_sb = const.tile([128, H, 2, 2, 128], f16, tag="wtop")
        wcc_sb = const.tile([128, 2 * H, OUTF], f16, tag="wcc")

        comb_sb = pk_sb[:, 0:8]

        def pblk_sl(idx):  # pblk[:, idx, :] from packed cols [8:72]
            return pk_sb[:, 8 + idx * 4:8 + idx * 4 + 4]

        def d1t_sl(k):  # d1t[:, k, :] from packed cols [72:104]
            return pk_sb[:, 72 + k * BLOC:72 + (k + 1) * BLOC]

        id8_sb = pk_sb[0:8, 104:112]
        ones18_sb = pk_sb[0:1, 112:120]
        bcc_sb = pk_sb[0:1, 120:248]

        # sync: w1/xt0 interleaved by kf-halves (first d3 matmuls can
        # start before the rest of the data lands) -> packed -> b1c -> rest
        nc.sync.dma_start(out=w1_sb[:, 0:2, :, :], in_=w1_d[:, 0:2, :, :])
        nc.sync.dma_start(out=xt_sb[:, 0:2, 0, :], in_=xt_d[:, 0:2, 0, :])
        nc.sync.dma_start(out=w1_sb[:, 2:4, :, :], in_=w1_d[:, 2:4, :, :])
        nc.sync.dma_start(out=xt_sb[:, 2:4, 0, :], in_=xt_d[:, 2:4, 0, :])
        nc.sync.dma_start(out=pk_sb, in_=pk_d[:, :])
        nc.sync.dma_start(out=b1c_sb, in_=b1c_d[:, :])
        nc.sync.dma_start(out=wbot_sb, in_=wbot_d[:, :, :, :, :])
        nc.sync.dma_start(out=xt_sb[:, :, 1:4, :], in_=xt_d[:, :, 1:4, :])
        nc.sync.dma_start(out=xt_sb[:, :, 4:8, :], in_=xt_d[:, :, 4:8, :])
        nc.sync.dma_start(out=wcc_sb, in_=wcc_d[:, :, :])
        # scalar (2nd HWDGE queue): wv -> wtop
        nc.scalar.dma_start(out=wv_sb, in_=wv_d[:, :, :])
        nc.scalar.dma_start(out=wtop_sb, in_=wtop_d[:, :, :, :, :])

        d4t_sb = const.tile([128, 2, BLOC], f16, tag="d4t")

        def emit_d4():
            pd4 = psmall.tile([128, 2, BLOC], f32, tag="small")
            for m in range(2):
                for k in range(4):
                    nc.tensor.matmul(pd4[:, m, :], lhsT=w1_sb[:, k, m, :],
                                     rhs=d1t_sl(k),
                                     start=(k == 0), stop=(k == 3))
            for m in range(2):
                nc.scalar.activation(d4t_sb[:, m, :], pd4[:, m, :], AF.Relu,
                                     bias=b1c_sb[:, m:m + 1])

        # ---- atts accumulator bank: zero once (garbage rows stay 0) ----
        pat = patp.tile([128, S], f32, tag="atts")
        nc.vector.memset(pat[:, :], 0.0)

        v_sb = vpool.tile([128, 2, H, BLOC], f16)  # [c-in-half, ch, h, b]
        zc_sb = const.tile([128, 2, H, BLOC], f32, tag="zc")

        # ---- pipeline stage emitters ----
        d3ts = [None] * BLOC
        tvs = [None] * BLOC
        uss = {}
        pat2s = [None] * BLOC
        nmaxs = [None] * BLOC
        escs = [None] * BLOC
        zinvs = [None] * BLOC
        scts = [None] * BLOC
        vssbs = [None] * BLOC

        def emit_d3_tv(b):
            xt = xt_sb[:, :, b, :]
            pmd3 = pmm.tile([128, 2, S], f32, tag="mm", name=f"pmd3_{b}")
            for m in range(2):
                for kf in range(4):
                    nc.tensor.matmul(pmd3[:, m, :], lhsT=w1_sb[:, kf, m, :],
                                     rhs=xt[:, kf, :],
                                     start=(kf == 0), stop=(kf == 3))
            d3t = d3p.tile([128, 2, S], f16, tag="d3t", name=f"d3t{b}")
            for m in range(2):
                nc.vector.tensor_scalar(
                    d3t[:, m, :], pmd3[:, m, :],
                    scalar1=b1c_sb[:, m:m + 1], scalar2=0.0,
                    op0=ALU.add, op1=ALU.max)
            d3ts[b] = d3t
            pmtv = pmm.tile([128, 4, C], f32, tag="mm", name=f"pmtv_{b}")
            for sc in range(4):
                for kf in range(4):
                    nc.tensor.matmul(
                        pmtv[:, sc, :],
                        lhsT=xt[:, kf, sc * 128:(sc + 1) * 128],
                        rhs=wv_sb[:, kf, :],
                        start=(kf == 0), stop=(kf == 3))
            tv = tvpool.tile([128, 4, C], f16, tag="tv", name=f"tv{b}")
            nc.scalar.activation(tv[:, :, :], pmtv[:, :, :], AF.Tanh)
            tvs[b] = tv
            if debug and b == dbg_b:
                nc.sync.dma_start(out=dbg_d3t[:, :, :], in_=d3t)
                nc.sync.dma_start(out=dbg_tv[:, :, :], in_=tv)

        def emit_zc():
            pzc = psmall.tile([128, 2, H, BLOC], f32, tag="small")
            for ct in range(2):
                for h in range(H):
                    for ks in range(2):
                        nc.tensor.matmul(pzc[:, ct, h, :],
                                         lhsT=wbot_sb[:, h, ks, ct, :],
                                         rhs=d4t_sb[:, ks, :],
                                         start=(ks == 0), stop=(ks == 1))
            nc.vector.tensor_copy(out=zc_sb, in_=pzc)
            if debug:
                nc.sync.dma_start(out=dbg_zc[:, :, :, :], in_=zc_sb)

        def emit_zs(b, h):
            pz = pzs.tile([128, 2, S], f32, tag="zs", name=f"pz{b}_{h}")
            for ct in range(2):
                for ks in range(2):
                    nc.tensor.matmul(pz[:, ct, :],
                                     lhsT=wtop_sb[:, h, ks, ct, :],
                                     rhs=d3ts[b][:, ks, :],
                                     start=(ks == 0), stop=(ks == 1))
            us = usp.tile([128, 2, S], f16, tag="us", name=f"us{b}_{h}")
            for ct in range(2):
                nc.scalar.activation(us[:, ct, :], pz[:, ct, :], AF.Tanh,
                                     bias=zc_sb[:, ct, h, b:b + 1])
            uss[(b, h)] = us

        def emit_pat_quad(b, k):
            # 4 adjacent matmuls covering all 4 col-groups -> concurrent.
            for hh in (2 * k, 2 * k + 1):
                us = uss.pop((b, hh))
                for ct in range(2):
                    g = 2 * (hh % 2) + ct
                    # per-col-group accumulation chains; start=True clears
                    # has_written only for the addressed partition rows.
                    nc.tensor.matmul(pat[32 * g:32 * g + 4, :],
                                     lhsT=pblk_sl(hh * 2 + ct),
                                     rhs=us[:, ct, :],
                                     start=(hh < 2), stop=(hh >= 6),
                                     skip_group_check=True,
                                     tile_position=(0, 32 * g))

        def emit_softmax_pre(b):
            attsg = smallsb.tile([128, S], f16, tag="attsg", name=f"ag{b}")
            nc.vector.tensor_copy(out=attsg, in_=pat)
            if debug and b == dbg_b:
                nc.sync.dma_start(out=dbg_attsg[:, :], in_=attsg)
            pat2 = psmall.tile([8, S], f32, tag="small", name=f"pat2_{b}")
            nc.tensor.matmul(pat2, lhsT=comb_sb, rhs=attsg,
                             start=True, stop=True)
            nmax = smallsb.tile([8, 1], f32, tag="nmax", name=f"nmax{b}")
            nc.vector.tensor_reduce(nmax, pat2, axis=mybir.AxisListType.X,
                                    op=ALU.max, negate=True)
            pat2s[b] = pat2
            nmaxs[b] = nmax

        def emit_softmax_exp(b):
            esc = smallsb.tile([8, S], f16, tag="esc", name=f"esc{b}")
            zsum = smallsb.tile([8, 1], f32, tag="zsum", name=f"zsum{b}")
            nc.scalar.activation(esc, pat2s[b], AF.Exp, bias=nmaxs[b],
                                 accum_out=zsum)
            zinv = smallsb.tile([8, 1], f32, tag="zinv", name=f"zinv{b}")
            nc.vector.reciprocal(zinv, zsum)
            escs[b] = esc
            zinvs[b] = zinv
            if debug and b == dbg_b:
                nc.sync.dma_start(out=dbg_esc[:, :], in_=esc)

        def emit_tail_psc(b):
            psc = psmall.tile([128, 4, 8], f16, tag="small", name=f"psc{b}")
            for sc in range(4):
                nc.tensor.transpose(psc[:, sc, :],
                                    in_=escs[b][:, sc * 128:(sc + 1) * 128],
                                    identity=id8_sb)
            sct = smallsb.tile([128, 4, 8], f16, tag="sct", name=f"sct{b}")
            nc.vector.tensor_copy(out=sct, in_=psc)
            scts[b] = sct

        def emit_tail_vs(b):
            pvs = psmall.tile([8, C], f32, tag="small", name=f"pvs{b}")
            for sc in range(4):
                nc.tensor.matmul(pvs, lhsT=scts[b][:, sc, :],
                                 rhs=tvs[b][:, sc, :],
                                 start=(sc == 0), stop=(sc == 3))
            vssb = smallsb.tile([8, C], f16, tag="vssb", name=f"vssb{b}")
            nc.vector.tensor_scalar_mul(vssb, pvs, zinvs[b])
            vssbs[b] = vssb
            if debug:
                nc.sync.dma_start(out=dbg_vs[b, :, :], in_=vssb)

        def emit_tail_pvt(b):
            pvt = psmall.tile([128, 2, 8], f16, tag="small", name=f"pvt{b}")
            for ch in range(2):
                nc.tensor.transpose(
                    pvt[:, ch, :],
                    in_=vssbs[b][:, ch * 128:(ch + 1) * 128],
                    identity=id8_sb)
            for ch in range(2):
                nc.vector.tensor_copy(out=v_sb[:, ch, :, b:b + 1],
                                      in_=pvt[:, ch, :])

        # ---- pipelined emission ----
        emit_d3_tv(0)
        emit_d4()
        emit_zc()
        for b in range(BLOC):
            for h in range(H):
                emit_zs(b, h)
                if h == 2:
                    emit_pat_quad(b, 0)
                if h == 4:
                    emit_pat_quad(b, 1)
                if h == 6:
                    emit_pat_quad(b, 2)
                if b > 0:
                    if h == 0:
                        emit_softmax_pre(b - 1)
                    elif h == 1:
                        emit_softmax_exp(b - 1)
                    elif h == 2:
                        emit_tail_psc(b - 1)
                    elif h == 3:
                        emit_tail_vs(b - 1)
                    elif h == 4:
                        emit_tail_pvt(b - 1)
            if b + 1 < BLOC:
                emit_d3_tv(b + 1)
            emit_pat_quad(b, 3)
        emit_softmax_pre(BLOC - 1)
        emit_softmax_exp(BLOC - 1)
        emit_tail_psc(BLOC - 1)
        emit_tail_vs(BLOC - 1)
        emit_tail_pvt(BLOC - 1)

        # ---- final: out = relu(V.T @ wcc + bcc) ----
        pout = psmall.tile([8, OUTF], f32, tag="small")
        kidx = 0
        for h in range(H):
            for ch in range(2):
                nc.tensor.matmul(pout, lhsT=v_sb[:, ch, h, :],
                                 rhs=wcc_sb[:, h * 2 + ch, :],
                                 start=(kidx == 0), stop=False)
                kidx += 1
        nc.tensor.matmul(pout, lhsT=ones18_sb, rhs=bcc_sb,
                         start=False, stop=True)
        outsb = smallsb.tile([8, OUTF], f32, tag="outsb")
        nc.scalar.activation(outsb, pout, AF.Relu)
        nc.sync.dma_start(out=out_d[:, :], in_=outsb)

    nc.compile()
    return nc


def host_inputs(d1, d2, w1, b1, W, P, wv, wcc, bcc):
    """Host-side sharding + layout prep. Returns in_maps for 8 cores."""
    d1 = np.ascontiguousarray(d1, dtype=np.float32)
    d2 = np.ascontiguousarray(d2, dtype=np.float32)
    w1 = np.ascontiguousarray(w1, dtype=np.float32)
    b1 = np.ascontiguousarray(b1, dtype=np.float32)
    W = np.ascontiguousarray(W, dtype=np.float32)
    P = np.ascontiguousarray(P, dtype=np.float32)
    wv = np.ascontiguousarray(wv, dtype=np.float32)
    wcc = np.ascontiguousarray(wcc, dtype=np.float32)
    bcc = np.ascontiguousarray(bcc, dtype=np.float32)

    w1r = np.ascontiguousarray(
        w1.reshape(4, 128, 2, 128).transpose(1, 0, 2, 3))
    wvr = np.ascontiguousarray(wv.reshape(4, 128, C).transpose(1, 0, 2))
    wtopr = np.ascontiguousarray(
        W[:, :C, :].reshape(H, 2, 128, 2, 128).transpose(2, 0, 1, 3, 4))
    wbotr = np.ascontiguousarray(
        W[:, C:, :].reshape(H, 2, 128, 2, 128).transpose(2, 0, 1, 3, 4))
    # 4-way col-tiled P blocks: head h, half ct -> col-group g=2*(h%2)+ct,
    # output row r=h//2 within the group.
    pblkr = np.zeros((128, 2 * H, 4), np.float32)
    combr = np.zeros((128, H), np.float32)
    for h in range(H):
        r = h // 2
        for ct in range(2):
            g = 2 * (h % 2) + ct
            pblkr[:, h * 2 + ct, r] = P[h, ct * 128:(ct + 1) * 128]
            combr[32 * g + r, h] = 1.0
    wccr = np.ascontiguousarray(
        wcc.reshape(2 * H, 128, OUTF).transpose(1, 0, 2))
    bccr = np.ascontiguousarray(bcc[None, :])
    b1c = np.ascontiguousarray(b1.reshape(2, 128).T)
    id8 = np.eye(8, dtype=np.float32)
    ones18 = np.ones((1, 8), np.float32)

    f16 = np.float16
    packed0 = np.zeros((128, 248), np.float32)
    packed0[:, 0:8] = combr
    packed0[:, 8:72] = pblkr.reshape(128, 64)
    packed0[0:8, 104:112] = id8
    packed0[0:1, 112:120] = ones18
    packed0[0:1, 120:248] = bccr
    shared = dict(w1r=w1r.astype(f16), wvr=wvr.astype(f16),
                  wtopr=wtopr.astype(f16), wbotr=wbotr.astype(f16),
                  wccr=wccr.astype(f16), b1c=b1c)
    in_maps = []
    for core in range(NCORES):
        bs = slice(core * BLOC, (core + 1) * BLOC)
        # xt[p, kf, b, s] = d2[s, bs.start+b, kf*128+p]
        d2c = d2[:, bs, :]  # [S, BLOC, F]
        xtr = np.ascontiguousarray(
            d2c.transpose(2, 1, 0).reshape(4, 128, BLOC, S)
            .transpose(1, 0, 2, 3)).astype(np.float16)
        d1c = d1[bs]  # [BLOC, F]
        d1tr = d1c.T.reshape(4, 128, BLOC).transpose(1, 0, 2)
        packed = packed0.copy()
        packed[:, 72:104] = d1tr.reshape(128, 32)
        in_maps.append(dict(xt=xtr, packed=packed.astype(f16), **shared))
    return in_maps


def kernel(**inputs):
    if "nc" not in _CACHE:
        _CACHE["nc"] = build_nc()
    nc = _CACHE["nc"]
    in_maps = host_inputs(
        d1=inputs["d1"], d2=inputs["d2"], w1=inputs["w1"], b1=inputs["b1"],
        W=inputs["W"], P=inputs["P"], wv=inputs["wv"], wcc=inputs["wcc"],
        bcc=inputs["bcc"])
    from concourse.bass_utils import run_bass_kernel_spmd
    res = run_bass_kernel_spmd(nc, in_maps, core_ids=list(range(NCORES)))
    return np.concatenate([res.results[i]["out"] for i in range(NCORES)],
                          axis=0)
